# revision 17
# baseline (speedup 1.0000x reference)
"""LQLinear (2-bit learned VQ linear) Trainium2 kernel.

Math (Q_T=1): the least-squares basis refit only feeds the *discarded*
buffer update, so the forward output is

    out = x @ wq.T + bias

where wq bucketizes weight into the 4 sorted levels {+-b_small +- b_big}
(b_small, b_big = sorted |basis|), thresholds at midpoints {-b_big, 0, +b_big}.

Device strategy (8 cores, out_features-sharded, 512 rows each):
  - wq = b_small * wqn with wqn in {+-1, +-3} for the reference basis
    (b_big = 2*b_small): EXACT in fp8e4.
  - greedy sign quantization == bucketize, decided in f32: s_big = sign(w),
    ss2 = sign(w^2 - b_big^2), wqn = s_big * (2 + ss2).
  - GEMM: ALL matmuls are fp8e4m3 DoubleRow (256 k rows per MM, ~241ns at
    N=512 = 578 cyc @2.4GHz):
      * 16 main pairs cover all 4096 k with x cast to e4m3 (rel err of
        e4m3-x alone: 2.32e-2, over the 2e-2 gate), plus
      * NHL=5 residual-correction pairs re-running k 0..1279 with moving
        x_c = e4m3(x - e4m3(x)) and the SAME wq stationary tiles, which
        cancels those dims' quantization error (exact host-checked rel
        err 1.922e-2; NHL=6 gives 1.832e-2).
    CRITICAL clock finding: mixing DR and normal-mode matmuls in one NEFF
    drops the WHOLE core clock 2.4 -> 2.0 GHz (every engine slows 1.2x,
    measured), so a bf16 k-split loses to all-DR + redundant correction
    pairs even though corrections redo work. All-DR keeps 2.4 GHz.
    MM floor: 64 groups x 21 x 241ns ~= 324us vs 437us for bf16-only.
  - Quantize is pipelined per k-tile; tb=0 consumes pairs in bursts as they
    emerge (correction MM for pair j issues right after main MM j since the
    stationary tile is shared).
  - w-loads interleave ahead of x tb-fetches in groups of 8 on the SAME
    (sync) HW-DGE ring (separate rings starve the 2KB w packets behind the
    big x packets). Out-stores use the scalar ring.
  - DVE evicts PSUM with fused out = b_small*psum + bias[o].
  - Host prep is layout-only sharding work (transpose/cast/block).

Measured NOT to help in prior sessions: bf16/fp8-DR mixed k-split (406us
— clock throttled), fp8 e3m4 DoubleRow (rejected by walrus
checkMatmultPerfMode), full hi+lo on all k (2x DR work, 494us),
interleaving 2 token blocks across all 8 psum banks, HAM warmup MMs.
"""

import os
import sys

for _p in ("/opt/trn_rl_repo", "/root/.axon_site/_ro/trn_rl_repo"):
    if os.path.isdir(_p) and _p not in sys.path:
        sys.path.insert(0, _p)

import numpy as np
import ml_dtypes

N_CORES = 8
TOKENS = 8192
IN_F = 4096
OUT_F = 4096
O_SHARD = OUT_F // N_CORES          # 512 output rows per core
KT = IN_F // 128                    # 32 k-tiles
NP_MAIN = KT // 2                   # 16 DoubleRow pairs cover all k
TB = 512                            # token block (psum free dim)
N_TB = TOKENS // TB                 # 16 token blocks
O_SUB = O_SHARD // 128              # 4 output subtiles per core

NHL = int(os.environ.get("LQ_NHL", "5"))   # residual-correction pairs
KC = 256 * NHL                      # corrected k-dims

LAST_RUN_INFO = {}


def _build_nc(b_small: float, b_big: float):
    import concourse.mybir as mybir
    import concourse.tile as tile
    from concourse import bacc

    dt = mybir.dt
    Alu = mybir.AluOpType
    DR = mybir.MatmulPerfMode.DoubleRow

    R = b_big / b_small

    nc = bacc.Bacc("TRN2", target_bir_lowering=False,
                   debug=os.environ.get("LQ_DEBUG", "0") == "1")

    # blocked, fully-contiguous-per-partition host layouts
    wT = nc.dram_tensor("wT", [KT, 128, O_SHARD], dt.float32, kind="ExternalInput")
    xdr = nc.dram_tensor("xdr", [N_TB, 128, NP_MAIN, 2, TB], dt.float8e4,
                         kind="ExternalInput")
    xc = None
    if NHL:
        xc = nc.dram_tensor("xc", [N_TB, 128, NHL, 2, TB], dt.float8e4,
                            kind="ExternalInput")
    bs = nc.dram_tensor("bs", [128, O_SUB], dt.float32, kind="ExternalInput")
    oT = nc.dram_tensor("oT", [N_TB, O_SUB, 128, TB], dt.bfloat16,
                        kind="ExternalOutput")

    wT_r = wT.ap()                  # [kt][128, 512]
    xdr_r = xdr.ap()                # [tb][128, 16, 2, 512]
    xc_r = xc.ap() if xc is not None else None  # [tb][128, NHL, 2, 512]
    oT_r = oT.ap()                  # [tb][osb][128, 512]

    with tile.TileContext(nc) as tc:
        with (
            tc.tile_pool(name="const", bufs=1) as const,
            tc.tile_pool(name="wq", bufs=1) as wqp,
            tc.tile_pool(name="wload", bufs=8) as wload,
            tc.tile_pool(name="quant", bufs=4) as qp,
            tc.tile_pool(name="xdrp", bufs=4) as xdrp,
            tc.tile_pool(name="xcp", bufs=4) as xcp,
            tc.tile_pool(name="outp", bufs=8) as outp,
            tc.tile_pool(name="psum", bufs=8, space="PSUM") as psp,
        ):
            bias_sb = const.tile([128, O_SUB], dt.float32)
            nc.sync.dma_start(bias_sb[:], bs.ap())
            nbb2 = const.tile([128, 1], dt.float32, tag="nbb2")
            nc.vector.memset(nbb2[:], -float(np.float32(b_big) * np.float32(b_big)))
            rcon = const.tile([128, 1], dt.float32, tag="rcon")
            nc.vector.memset(rcon[:], R)

            # persistent quantized-weight DR pair tiles (shared by main and
            # correction matmuls)
            wq_dr = [wqp.tile([128, 2, O_SHARD], dt.float8e4, tag=f"wqdr{j}",
                              name=f"wqdr{j}")
                     for j in range(NP_MAIN)]

            x_tiles = {}
            HP = NP_MAIN // 2       # pairs per xd half-tile

            def fetch_xa(tb):
                xa = xdrp.tile([128, HP, 2, TB], dt.float8e4,
                               tag="xa", name="xa")
                nc.sync.dma_start(xa[:], xdr_r[tb][:, :HP])
                x_tiles[(tb, "a")] = xa

            def fetch_xb(tb):
                xb = xdrp.tile([128, HP, 2, TB], dt.float8e4,
                               tag="xb", name="xb")
                nc.sync.dma_start(xb[:], xdr_r[tb][:, HP:])
                x_tiles[(tb, "b")] = xb

            def fetch_xc(tb):
                if not NHL:
                    x_tiles[(tb, "c")] = None
                    return
                xct = xcp.tile([128, NHL, 2, TB], dt.float8e4,
                               tag="xc", name="xc")
                nc.sync.dma_start(xct[:], xc_r[tb])
                x_tiles[(tb, "c")] = xct

            def fetch_x(tb):
                fetch_xa(tb)
                fetch_xb(tb)
                fetch_xc(tb)

            def pop_x(tb):
                return (x_tiles.pop((tb, "a")), x_tiles.pop((tb, "b")),
                        x_tiles.pop((tb, "c")))

            # ---- quantize weight shard -> wqn {+-1,+-R} fp8, one tile per kt
            def quantize_w(kt):
                w_t = wload.tile([128, O_SHARD], dt.float32, tag="wl")
                nc.sync.dma_start(w_t[:], wT_r[kt])
                sb = qp.tile([128, O_SHARD], dt.float32, tag="sb")
                av = qp.tile([128, O_SHARD], dt.float32, tag="av")
                # ss2 = sign(|w| - b_big) computed as sign(w^2 - b_big^2)
                # (w^2 on DVE so ACT only does 2 ops per k-tile)
                nc.vector.tensor_tensor(av[:], w_t[:], w_t[:], Alu.mult)
                nc.scalar.sign(sb[:], w_t[:])
                nc.scalar.sign(av[:], av[:], bias=nbb2[:])
                # wqn = s_big * (R + ss2); the +R alternates ACT/DVE per
                # k-tile to balance both engines through the quantize window
                if kt % 2 == 0:
                    nc.vector.tensor_scalar(av[:], av[:], R, None, Alu.add)
                else:
                    nc.scalar.activation(av[:], av[:],
                                         mybir.ActivationFunctionType.Identity,
                                         rcon[:])
                nc.vector.tensor_tensor(wq_dr[kt // 2][:, kt % 2, :],
                                        sb[:], av[:], Alu.mult)

            # Issue order on the sync ring is FIFO. tb0 AND tb1 are both
            # processed during the quantize window (all 8 psum banks) so the
            # PE has ~36us of work to overlap the ~45us w/x0/x1 delivery;
            # x half-tiles interleave into the w stream just-in-time, x(2+)
            # go after the last w tile so the final pairs aren't delayed.
            quantize_w(0)
            quantize_w(1)
            fetch_xa(0)
            for kt in range(2, 8):
                quantize_w(kt)
            fetch_xa(1)
            for kt in range(8, 16):
                quantize_w(kt)
            fetch_xb(0)
            fetch_xb(1)
            for kt in range(16, 24):
                quantize_w(kt)
            fetch_xc(0)
            fetch_xc(1)
            for kt in range(24, KT):
                quantize_w(kt)
            fetch_x(2)
            fetch_x(3)

            def mm(ps, osb, j, xab, start, stop):
                x_t = xab[0] if j < HP else xab[1]
                nc.tensor.matmul(
                    ps[:], wq_dr[j][:, :, osb * 128:(osb + 1) * 128],
                    x_t[:, j % HP, :, :], start=start, stop=stop, perf_mode=DR)

            def mmc(ps, osb, j, xct, start, stop):
                nc.tensor.matmul(
                    ps[:], wq_dr[j][:, :, osb * 128:(osb + 1) * 128],
                    xct[:, j, :, :], start=start, stop=stop, perf_mode=DR)

            def evict(tb, osb, ps):
                o_t = outp.tile([128, TB], dt.bfloat16, tag="ot")
                # out = b_small * psum + bias  (per-partition bias AP)
                nc.vector.tensor_scalar(o_t[:], ps[:], float(b_small),
                                        bias_sb[:, osb:osb + 1],
                                        Alu.mult, Alu.add)
                nc.scalar.dma_start(oT_r[tb, osb], o_t[:])

            # ---- GEMM  psum[o128, t512] += wq[k,o].T @ x[k,t], all DR MMs.
            # tb0+tb1 warmup: consume main pairs in bursts of 2 (pair j's
            # weights land at kt 2j+1) across all 8 psum banks, corrections
            # last (their xc tiles arrive near the end of the w stream).
            x01 = [pop_x(0), pop_x(1)]
            ps01 = [[psp.tile([128, TB], dt.float32, tag="ps",
                              name=f"ps{t}{osb}")
                     for osb in range(O_SUB)] for t in range(2)]
            for burst_j in range(0, NP_MAIN, 2):
                for t in range(2):
                    for osb in range(O_SUB):
                        for j in (burst_j, burst_j + 1):
                            mm(ps01[t][osb], osb, j, x01[t],
                               start=(j == 0),
                               stop=(NHL == 0 and j == NP_MAIN - 1))
            for t in range(2):
                for osb in range(O_SUB):
                    for j in range(NHL):
                        mmc(ps01[t][osb], osb, j, x01[t][2],
                            start=False, stop=(j == NHL - 1))
                    evict(t, osb, ps01[t][osb])

            for tb in range(2, N_TB):
                if tb + 2 < N_TB:
                    fetch_x(tb + 2)
                xab = pop_x(tb)
                for osb in range(O_SUB):
                    ps = psp.tile([128, TB], dt.float32, tag="ps", name="ps")
                    for j in range(NP_MAIN):
                        mm(ps, osb, j, xab, start=(j == 0),
                           stop=(NHL == 0 and j == NP_MAIN - 1))
                    for j in range(NHL):
                        mmc(ps, osb, j, xab[2], start=False,
                            stop=(j == NHL - 1))
                    if tb == N_TB - 1:
                        # last tb: half-column evict/store slices so the
                        # stores overlap the evictions in the drain tail
                        o_t = outp.tile([128, TB], dt.bfloat16, tag="ot",
                                        name="ot_tail")
                        for half in range(2):
                            sl = slice(half * (TB // 2), (half + 1) * (TB // 2))
                            nc.vector.tensor_scalar(
                                o_t[:, sl], ps[:, sl], float(b_small),
                                bias_sb[:, osb:osb + 1], Alu.mult, Alu.add)
                            nc.scalar.dma_start(oT_r[tb, osb][:, sl],
                                                o_t[:, sl])
                    else:
                        evict(tb, osb, ps)

    nc.compile()
    return nc


def kernel(x, weight, bias, basis):
    from concourse import bass_utils

    x = np.asarray(x, dtype=np.float32)
    weight = np.asarray(weight, dtype=np.float32)
    bias = np.asarray(bias, dtype=np.float32)
    basis = np.asarray(basis, dtype=np.float32)

    b_small, b_big = sorted(float(v) for v in np.abs(basis))

    # ---- host-side shard/layout prep (transpose, cast, block)
    # xdr[tb, p, j, i, t] = e4m3(x[tb*512+t, (2j+i)*128+p])
    # xc[tb, p, j, i, t]  = e4m3((x - e4m3(x))[tb*512+t, (2j+i)*128+p]), j<NHL
    f8 = ml_dtypes.float8_e4m3
    x8 = x.astype(f8)
    xdr = np.ascontiguousarray(
        x8.reshape(N_TB, TB, NP_MAIN, 2, 128).transpose(0, 4, 2, 3, 1))
    xlo8 = (x[:, :KC] - x8[:, :KC].astype(np.float32)).astype(f8)
    xcb = np.ascontiguousarray(
        xlo8.reshape(N_TB, TB, NHL, 2, 128).transpose(0, 4, 2, 3, 1)) \
        if NHL else None
    wt = weight.T                                        # [4096 in, 4096 out]

    in_maps = []
    for c in range(N_CORES):
        wb = np.ascontiguousarray(
            wt[:, c * O_SHARD:(c + 1) * O_SHARD]).reshape(KT, 128, O_SHARD)
        m = {
            "wT": wb,
            "xdr": xdr,
            "bs": np.ascontiguousarray(
                bias[c * O_SHARD:(c + 1) * O_SHARD].reshape(O_SUB, 128).T),
        }
        if NHL:
            m["xc"] = xcb
        in_maps.append(m)

    nc = _build_nc(b_small, b_big)
    trace = os.environ.get("LQ_TRACE", "") == "1"

    # random-projection ground truth for readback validation: catches
    # transport-level corruption (e.g. output blocks read back before the
    # final stores land). Expectation includes the fp8 casts of x, so the
    # residual only contains device arithmetic noise (fp32 psum order,
    # DoubleRow per-cell rounding ~1e-4).
    rng = np.random.default_rng(12345)
    v = rng.standard_normal(OUT_F)
    wqn = np.sign(weight) * (2.0 + np.sign(weight * weight
                                           - np.float32(b_big) ** 2))
    u = (b_small * wqn.astype(np.float64)).T @ v          # [IN_F]
    r_exp = x8.astype(np.float64) @ u + float(bias @ v)
    if NHL:
        r_exp = r_exp + xlo8.astype(np.float64) @ u[:KC]
    r_scale = np.linalg.norm(r_exp)

    out = None
    for attempt in range(3):
        try:
            res = bass_utils.run_bass_kernel_spmd(
                nc, in_maps, core_ids=list(range(N_CORES)), trace=trace)
        except Exception:
            if attempt == 2:
                raise
            continue

        LAST_RUN_INFO.clear()
        LAST_RUN_INFO["exec_time_ns"] = res.exec_time_ns
        LAST_RUN_INFO["profile_json"] = res.profile_json
        LAST_RUN_INFO["nc"] = nc
        LAST_RUN_INFO["in_maps"] = in_maps

        # oT blocked [tb, osb, p, t]: rows osb*128+p of shard, cols tb*512+t
        outT = np.concatenate(
            [res.results[c]["oT"].transpose(1, 2, 0, 3).reshape(O_SHARD, TOKENS)
             for c in range(N_CORES)], axis=0)
        out = np.ascontiguousarray(outT.T).astype(np.float32)

        resid = np.linalg.norm(out.astype(np.float64) @ v - r_exp) / r_scale
        if resid < 5e-3:
            break
    return out
